# revision 2
# baseline (speedup 1.0000x reference)
"""Conv4d (3,3,3,3) kernel for Trainium2, 8 NeuronCores.

Problem: x (2,24,16,16,48,48) * weight (48,24,3,3,3,3) + bias3d.sum(0)
      -> out (2,48,14,14,46,46), stride 1, no padding.

Strategy
--------
Sharding: 8 cores = (batch 2) x (ol-block 2) x (od-block 2); each core owns a
7x7 block of (ol, od) output planes (49 tasks). Per task: implicit GEMM with
contraction rows (lo,do,ci) = 216 (+1 ones row for bias) packed on the host;
the 9 (ho,wo) kernel offsets are free-dim shifts of the same SBUF tile, all
accumulating into one PSUM bank per output-row chunk.

Performance structure (NTFF-verified on HW, core-0 span 444.9us vs 1005.5us
for the single-task predecessor):
- Dual-task column tiling: tasks are processed in PAIRS on the PE's 128x64
  col-tiled mode — task A -> tile (0,0), psum partitions 0..47; task B ->
  tile (0,64), psum partitions 64..111. Both matmuls stream concurrently
  (~196ns per N=460 slot, fp16), doubling effective PE throughput; LDWEIGHTS
  hides under the other tile's stream.
- 128-partition DMA rule: HBM->SBUF transfers only fan out across the 16
  SDMA engines when they have exactly 128 partitions (an 89-partition load
  lands on ONE engine at ~26GB/s and serializes the whole kernel). The
  k-split is therefore 0..127 and 89..216 (overlapping); weight rows for the
  overlap (k2 rows 0..38) are zero, which costs no PE time (matmul time is
  N-cycles, K-independent).
- Input layout xs[krow, task, h, w] so a pair's 2 tasks load as one DMA with
  9.2KB/partition descriptors on the SP HWDGE ring (never nc.scalar: the ACT
  ring drains at single-engine rate).
- fp16 everywhere off-chip (PSUM accumulates fp32; epilogue DVE copy casts
  to fp16; host upcasts). rel err ~4.7e-4 vs fp32 reference.
"""

import os
import sys

if "/opt/trn_rl_repo" not in sys.path:
    sys.path.insert(0, "/opt/trn_rl_repo")

from contextlib import nullcontext

import numpy as np

from concourse import bacc, bass, tile
from concourse.bass_utils import run_bass_kernel_spmd

mybir = bass.mybir

B, CI, CO = 2, 24, 48
L, D, H, W = 16, 16, 48, 48
OL, OD, OH, OW = 14, 14, 46, 46
N_TASKS = 49  # 7x7 (ol, od) planes per core
KROWS = 217  # (lo,do,ci) contraction rows + ones row
KSPLIT = 128  # k1 = rows 0:128, k2 = rows 128:217

CHUNK_ROWS = (10, 10, 10, 10, 6)
CHUNK_OH0 = (0, 10, 20, 30, 40)

DTYPE = mybir.dt.float16
ODTYPE = mybir.dt.float16

X_BUFS = int(os.environ.get("CONV_XBUFS", "3"))
PS_BUFS = int(os.environ.get("CONV_PSBUFS", "8"))
O_BUFS = int(os.environ.get("CONV_OBUFS", "8"))


def _np_dtype():
    return mybir.dt.np(DTYPE)


def build_program(n_tasks: int = N_TASKS, repeat: int = 1):
    nc = bacc.Bacc()
    f32 = mybir.dt.float32
    # k2 covers global rows 89..216 (128 rows, overlapping k1's 89..127) so
    # the DMA is a full 128-partition transfer — 89-partition transfers do
    # NOT fan out across the 16 SDMA engines (all descriptors land on one
    # engine at ~26GB/s; measured root cause of the ~1ms DMA-bound plateau).
    # Correctness: w2 rows 0..38 (= global rows 89..127) are zero.
    k2rows = 128
    k2base = KROWS - 128  # 89
    nchunk = len(CHUNK_ROWS)

    # xs2: [KROWS, n_tasks, H, W] — tasks adjacent for a fixed contraction row
    xs_d = nc.dram_tensor("xs", [KROWS, n_tasks, H, W], DTYPE, kind="ExternalInput")
    out_d = nc.dram_tensor("out", [n_tasks, CO, OH, OW], ODTYPE, kind="ExternalOutput")
    w1_d = nc.dram_tensor("w1", [KSPLIT, 9, CO], DTYPE, kind="ExternalInput")
    w2_d = nc.dram_tensor("w2", [k2rows, 9, CO], DTYPE, kind="ExternalInput")

    pairs = [(t, t + 1 if t + 1 < n_tasks else None) for t in range(0, n_tasks, 2)]

    with tile.TileContext(nc) as tc:
        with (
            tc.tile_pool(name="wpool", bufs=1) as wpool,
            tc.tile_pool(name="xpool", bufs=X_BUFS) as xpool,
            tc.tile_pool(name="opool", bufs=O_BUFS) as opool,
            tc.tile_pool(name="pspool", bufs=PS_BUFS, space="PSUM") as pspool,
            tc.For_i(0, repeat, 1) if repeat > 1 else nullcontext(),
        ):
            w1s = wpool.tile([KSPLIT, 9, CO], DTYPE)
            w2s = wpool.tile([k2rows, 9, CO], DTYPE)
            nc.sync.dma_start(out=w1s[:], in_=w1_d[:])
            nc.sync.dma_start(out=w2s[:], in_=w2_d[:])

            for tA, tB in pairs:
                ntp = 1 if tB is None else 2
                k1 = xpool.tile([KSPLIT, ntp, H, W], DTYPE, tag="k1")
                k2 = xpool.tile([k2rows, ntp, H, W], DTYPE, tag="k2")
                # both input loads on the SP (sync) HWDGE ring: the ACT ring
                # drains at single-SDMA-engine rate (~24GB/s measured)
                nc.sync.dma_start(out=k1[:], in_=xs_d[0:KSPLIT, tA : tA + ntp])
                nc.sync.dma_start(out=k2[:], in_=xs_d[k2base:KROWS, tA : tA + ntp])

                ps_l = [
                    pspool.tile([128, 512], f32, tag="ps", name=f"ps{c}")
                    for c in range(nchunk)
                ]

                for idx in range(9):
                    ho, wo = divmod(idx, 3)
                    for kt, (ks, ws) in enumerate(((k1, w1s), (k2, w2s))):
                        wsl = ws[:, idx, :]
                        first = idx == 0 and kt == 0
                        last = idx == 8 and kt == 1
                        for c in range(nchunk):
                            rows = CHUNK_ROWS[c]
                            oh0 = CHUNK_OH0[c]
                            for j in range(ntp):
                                p0 = 64 * j
                                # rhs: [K, rows, 46] slice of task j's plane
                                rhs = ks[:, j, oh0 + ho : oh0 + ho + rows, wo : wo + OW]
                                nc.tensor.matmul(
                                    ps_l[c][p0 : p0 + CO, 0 : rows * OW],
                                    lhsT=wsl,
                                    rhs=rhs,
                                    start=first,
                                    stop=last,
                                    tile_position=(0, p0),
                                )

                for c in range(nchunk):
                    rows = CHUNK_ROWS[c]
                    oh0 = CHUNK_OH0[c]
                    o = opool.tile([128, CHUNK_ROWS[0], OW], ODTYPE, tag="o")
                    np_hi = 112 if ntp == 2 else CO
                    nc.vector.tensor_copy(
                        out=o[0:np_hi, :rows, :],
                        in_=ps_l[c][0:np_hi, 0 : rows * OW],
                    )
                    nc.gpsimd.dma_start(
                        out=out_d[tA, :, oh0 : oh0 + rows, :],
                        in_=o[0:CO, :rows, :],
                    )
                    if tB is not None:
                        nc.gpsimd.dma_start(
                            out=out_d[tB, :, oh0 : oh0 + rows, :],
                            in_=o[64 : 64 + CO, :rows, :],
                        )
    nc.finalize()
    return nc


def make_in_maps(x, weight, bias3d, n_tasks: int = N_TASKS):
    """Host-side shard + repack into [KROWS, n_tasks, H*W] layout."""
    npdt = _np_dtype()
    x = np.asarray(x, np.float32)
    weight = np.asarray(weight, np.float32)
    bias3d = np.asarray(bias3d, np.float32)

    # W[(lo*3+do)*24+ci, ho*3+wo, co] = weight[co, ci, lo, do, ho, wo]
    Wr = np.ascontiguousarray(np.transpose(weight, (2, 3, 1, 4, 5, 0))).reshape(
        216, 9, CO
    )
    Wfull = np.zeros((KROWS, 9, CO), np.float32)
    Wfull[:216] = Wr
    Wfull[216, 0, :] = bias3d.sum(axis=0)
    w1 = np.ascontiguousarray(Wfull[:KSPLIT]).astype(npdt)
    # w2 row p maps to global row 89+p; rows 89..127 already count in k1
    w2 = np.zeros((128, 9, CO), np.float32)
    w2[39:] = Wfull[KSPLIT:]
    w2 = w2.astype(npdt)

    in_maps = []
    for c in range(8):
        b, lb, db = c // 4, (c // 2) % 2, c % 2
        slab = np.ascontiguousarray(
            x[b, :, 7 * lb : 7 * lb + 9, 7 * db : 7 * db + 9]
        )  # (24, 9, 9, 48, 48)
        s_ci, s_l, s_d, s_h, s_w = slab.strides
        # V[l0, d0, lo, do, ci, h, w] = slab[ci, l0+lo, d0+do, h, w]
        V = np.lib.stride_tricks.as_strided(
            slab,
            shape=(7, 7, 3, 3, CI, H, W),
            strides=(s_l, s_d, s_l, s_d, s_ci, s_h, s_w),
        )
        xs = np.empty((KROWS, N_TASKS, H, W), np.float32)
        xs[:216] = V.reshape(N_TASKS, 216, H, W).transpose(1, 0, 2, 3)
        xs[216] = 1.0
        in_maps.append({"xs": xs[:, :n_tasks].astype(npdt), "w1": w1, "w2": w2})
    return in_maps


def assemble_output(results):
    out = np.empty((B, CO, OL, OD, OH, OW), np.float32)
    for c in range(8):
        b, lb, db = c // 4, (c // 2) % 2, c % 2
        r = np.asarray(results[c]["out"]).astype(np.float32).reshape(7, 7, CO, OH, OW)
        out[b, :, 7 * lb : 7 * lb + 7, 7 * db : 7 * db + 7] = r.transpose(2, 0, 1, 3, 4)
    return out


_NC_CACHE = {}


def _get_program():
    if "nc" not in _NC_CACHE:
        _NC_CACHE["nc"] = build_program()
    return _NC_CACHE["nc"]


def kernel(x, weight, bias3d):
    nc = _get_program()
    in_maps = make_in_maps(x, weight, bias3d)
    res = run_bass_kernel_spmd(nc, in_maps, list(range(8))).results
    return assemble_output(res)
